# revision 1
# baseline (speedup 1.0000x reference)
"""AttCML distributed Bass kernel for 8 TRN2 NeuronCores.

Sharding: data-parallel over the batch dim (16384 / 8 = 2048 per core).

The on-device toolchain here has no usable wide-index row gather
(indirect DMA is not lowered by this walrus pipeline; the Q7 dma_gather
ucode is int16-indexed), so kernel() performs the embedding-row lookup
host-side and ships packed per-core tensors; all attention compute
(scores, exp/normalize, weighted pooling, distances) runs on device.

Device-side structure per 128-batch tile (batch on partitions):
  - both targets (pos/neg) processed in single wide ops over a
    [128, 2, C, D] layout to halve instruction count
  - dot-product and pooling reductions are done as TT-add trees, which
    run at the DVE bf16 2x packed rate (native tensor_reduce runs 1x)
  - masked-out pref slots (position >= n+1) are never shipped: the host
    packs only cap[t] slots per tile and pads with the zero row; the
    kernel subtracts the pad count from the softmax denominator, and
    zero rows contribute nothing to the weighted sum — exact semantics.

Per-core batch layout: batch element order[p * NT + t] is at partition p,
tile t. The host unscrambles at the end.
"""

import numpy as np
from contextlib import ExitStack

try:
    import concourse  # noqa: F401
except ImportError:  # pragma: no cover
    import sys

    for _p in ("/opt/trn_rl_repo", "/root/.axon_site/_ro/trn_rl_repo"):
        if _p not in sys.path:
            sys.path.insert(0, _p)

import concourse.bacc as bacc
import concourse.tile as tile
from concourse import mybir
from concourse.bass_utils import run_bass_kernel_spmd

F32 = mybir.dt.float32
BF16 = mybir.dt.bfloat16
ALU = mybir.AluOpType
AXIS = mybir.AxisListType
ACTF = mybir.ActivationFunctionType

D = 128          # embedding dim
P = 50           # prefs per batch element
N_CORES = 8
B = 16384
BC = B // N_CORES  # 2048 batch per core
PB = 128           # batch tile = one SBUF partition set


def build_bass(bc: int = BC, cap=None):
    nt = bc // PB
    if cap is None:
        cap = (P,) * nt
    assert len(cap) == nt
    offs = [0]
    for c in cap:
        offs.append(offs[-1] + c)
    ctot = offs[-1]

    nc = bacc.Bacc(
        "TRN2",
        target_bir_lowering=False,
        debug=False,
        enable_asserts=False,
        num_devices=N_CORES,
    )

    # pref rows, bf16, host-packed: [PB, sum(cap) * D]
    pref_in = nc.declare_dram_parameter("pref", [PB, ctot * D], BF16, isOutput=False)
    # u vectors f32 [PB, nt*D]; pos/neg packed together [PB, nt*2*D]
    u_in = nc.declare_dram_parameter("uvec", [PB, nt * D], F32, isOutput=False)
    pn_in = nc.declare_dram_parameter("pnvec", [PB, nt * 2 * D], F32, isOutput=False)
    padc_in = nc.declare_dram_parameter("padc", [PB, nt], F32, isOutput=False)
    out = nc.declare_dram_parameter("out", [PB, 2 * nt], F32, isOutput=True)

    with tile.TileContext(nc) as tc, ExitStack() as ctx:
        consts = ctx.enter_context(tc.tile_pool(name="consts", bufs=1))
        pref_pool = ctx.enter_context(tc.tile_pool(name="pref", bufs=2))
        tmp_pool = ctx.enter_context(tc.tile_pool(name="tmpp", bufs=2))
        big_pool = ctx.enter_context(tc.tile_pool(name="big", bufs=1))
        vec_pool = ctx.enter_context(tc.tile_pool(name="vec", bufs=2))
        small_pool = ctx.enter_context(tc.tile_pool(name="small", bufs=3))

        padc = consts.tile([PB, nt], F32)
        nc.sync.dma_start(padc[:], padc_in[:])
        res = consts.tile([PB, 2 * nt], F32)

        for t in range(nt):
            C = cap[t]
            L = C * D
            pref = pref_pool.tile([PB, L], BF16, tag="pref")
            nc.sync.dma_start(pref[:], pref_in[:, offs[t] * D : offs[t + 1] * D])

            u_t = vec_pool.tile([PB, D], F32, tag="u")
            nc.sync.dma_start(u_t[:], u_in[:, t * D : (t + 1) * D])
            pn_t = vec_pool.tile([PB, 2 * D], F32, tag="pn")
            nc.sync.dma_start(pn_t[:], pn_in[:, t * 2 * D : (t + 1) * 2 * D])

            # bf16 copy of both targets (ACT)
            pn_b = vec_pool.tile([PB, 2 * D], BF16, tag="pnb")
            nc.scalar.copy(pn_b[:], pn_t[:])

            # ---- stage A: w[b, s, j] = pref[b, j, :] . tgt[b, s, :] ----
            # per-target ops keep the plain 3D broadcast form that measures
            # at the full 2x packed rate
            tmp = tmp_pool.tile([PB, 2 * L], BF16, tag="tmp")
            prefA = pref[:].rearrange("p (j d) -> p j d", d=D)
            for s in range(2):
                nc.vector.tensor_tensor(
                    out=tmp[:, s * L : (s + 1) * L].rearrange(
                        "p (j d) -> p j d", d=D
                    ),
                    in0=prefA,
                    in1=pn_b[:, s * D : (s + 1) * D]
                    .rearrange("p (o d) -> p o d", o=1)
                    .to_broadcast([PB, C, D]),
                    op=ALU.mult,
                )

            # tree-reduce over d at the TT 2x bf16 rate; short 1x tail
            cur, dd, lvl = tmp, D, 0
            while dd > 4:
                h = dd // 2
                nxt = big_pool.tile([PB, 2 * C * h], BF16, tag=f"ar{lvl}")
                lvl += 1
                nc.vector.tensor_tensor(
                    out=nxt[:].rearrange("p (k d) -> p k d", d=h),
                    in0=cur[:].rearrange("p (k d) -> p k d", d=dd)[:, :, :h],
                    in1=cur[:].rearrange("p (k d) -> p k d", d=dd)[:, :, h:],
                    op=ALU.add,
                )
                cur, dd = nxt, h
            w2 = small_pool.tile([PB, 2 * P], F32, tag="w2")
            nc.vector.tensor_reduce(
                out=w2[:, : 2 * C],
                in_=cur[:].rearrange("p (k d) -> p k d", d=dd),
                axis=AXIS.X,
                op=ALU.add,
            )

            # ---- stage B: att = exp(w) / (sum - padcount) ----
            e2 = small_pool.tile([PB, 2 * P], F32, tag="e2")
            nc.scalar.activation(e2[:, : 2 * C], w2[:, : 2 * C], ACTF.Exp)
            ssum = small_pool.tile([PB, 2], F32, tag="ssum")
            nc.vector.tensor_reduce(
                ssum[:],
                e2[:, : 2 * C].rearrange("p (s j) -> p s j", s=2),
                axis=AXIS.X,
                op=ALU.add,
            )
            scor = small_pool.tile([PB, 2], F32, tag="scor")
            nc.vector.tensor_tensor(
                scor[:],
                ssum[:],
                padc[:, t : t + 1].to_broadcast([PB, 2]),
                op=ALU.add,
            )
            rs = small_pool.tile([PB, 2], F32, tag="rs")
            nc.vector.reciprocal(rs[:], scor[:])
            att2 = small_pool.tile([PB, 2 * P], BF16, tag="att2")
            nc.vector.tensor_tensor(
                out=att2[:, : 2 * C].rearrange("p (s j) -> p s j", s=2),
                in0=e2[:, : 2 * C].rearrange("p (s j) -> p s j", s=2),
                in1=rs[:].rearrange("p (s o) -> p s o", o=1).to_broadcast(
                    [PB, 2, C]
                ),
                op=ALU.mult,
            )

            # ---- stage C: r[b, s, :] = sum_j att[b, s, j] * pref[b, j, :] ----
            # chunk expansion (ACT) + multiply (DVE) by target so the two
            # engines pipeline instead of DVE stalling on the full expansion
            attx = big_pool.tile([PB, 2 * L], BF16, tag="attx")
            tmp2 = big_pool.tile([PB, 2 * L], BF16, tag="tmp2")
            halves = [(0, C // 2), (C // 2, C)] if C >= 24 else [(0, C)]
            for s in range(2):
                for j0, j1 in halves:
                    cw = j1 - j0
                    lo = s * L + j0 * D
                    hi = s * L + j1 * D
                    attx_s = attx[:, lo:hi].rearrange("p (j d) -> p j d", d=D)
                    nc.scalar.copy(
                        attx_s,
                        att2[:, s * C + j0 : s * C + j1]
                        .rearrange("p (j o) -> p j o", o=1)
                        .to_broadcast([PB, cw, D]),
                    )
                    nc.vector.tensor_tensor(
                        out=tmp2[:, lo:hi].rearrange("p (j d) -> p j d", d=D),
                        in0=pref[:, j0 * D : j1 * D].rearrange(
                            "p (j d) -> p j d", d=D
                        ),
                        in1=attx_s,
                        op=ALU.mult,
                    )
            # tree-reduce over j (both targets at once), short strided 1x
            # tail on the final few chunks
            cur, cj, lvl = tmp2, C, 0
            while cj > 4:
                h = cj // 2
                odd = cj - 2 * h
                nxt = big_pool.tile([PB, 2 * (h + odd) * D], BF16, tag=f"cr{lvl}")
                lvl += 1
                nxt4 = nxt[:].rearrange("p (s j d) -> p s j d", s=2, d=D)
                cur4 = cur[:].rearrange("p (s j d) -> p s j d", s=2, d=D)
                nc.vector.tensor_tensor(
                    out=nxt4[:, :, :h, :],
                    in0=cur4[:, :, :h, :],
                    in1=cur4[:, :, h : 2 * h, :],
                    op=ALU.add,
                )
                if odd:
                    nc.vector.tensor_copy(
                        nxt4[:, :, h:, :], cur4[:, :, 2 * h : cj, :]
                    )
                cur, cj = nxt, h + odd
            r2 = vec_pool.tile([PB, 2 * D], F32, tag="r2")
            nc.vector.tensor_reduce(
                r2[:].rearrange("p (s d) -> p s d", s=2),
                cur[:, : 2 * cj * D].rearrange("p (s j d) -> p s d j", s=2, d=D),
                axis=AXIS.X,
                op=ALU.add,
            )

            # ---- distances: ||u + r - tgt||^2 ----
            du2 = vec_pool.tile([PB, 2 * D], F32, tag="du2")
            nc.vector.tensor_tensor(
                du2[:].rearrange("p (s d) -> p s d", s=2),
                u_t[:].rearrange("p (o d) -> p o d", o=1).to_broadcast(
                    [PB, 2, D]
                ),
                pn_t[:].rearrange("p (s d) -> p s d", s=2),
                op=ALU.subtract,
            )
            diff2 = vec_pool.tile([PB, 2 * D], F32, tag="diff2")
            nc.vector.tensor_add(diff2[:], r2[:], du2[:])
            # square + reduce entirely on ACT via accum_out (frees DVE)
            sq2 = vec_pool.tile([PB, 2 * D], F32, tag="sq2")
            for s in range(2):
                nc.scalar.activation(
                    sq2[:, s * D : (s + 1) * D],
                    diff2[:, s * D : (s + 1) * D],
                    ACTF.Square,
                    accum_out=res[:, s * nt + t : s * nt + t + 1],
                )

        nc.sync.dma_start(out[:], res[:])

    nc.compile()
    return nc


_CACHE: dict = {}


def _get_bass(bc: int, cap: tuple):
    key = (bc, cap)
    if key not in _CACHE:
        _CACHE[key] = build_bass(bc, cap)
    return _CACHE[key]


def prep_core(user_emb, ctx_item_bf16, ctx_item, user_ids, pos_ids, neg_ids,
              pref_ids, n_prefs, cap, order):
    """Build one core's input map.

    order: [bc] permutation; element order[p * nt + t] is placed at
    partition p, tile t.  Host guarantees n_prefs[order[p*nt+t]] + 1 <= cap[t].
    """
    bc = order.shape[0]
    nt = bc // PB
    offs = np.concatenate([[0], np.cumsum(cap)]).astype(np.int64)
    ctot = int(offs[-1])

    ob = order.reshape(PB, nt)
    n1 = (n_prefs[ob] + 1.0).astype(np.float32)  # [PB, nt] valid counts

    pref = np.zeros((PB, ctot, D), dtype=ctx_item_bf16.dtype)
    for t in range(nt):
        C = int(cap[t])
        ids_t = pref_ids[ob[:, t], :C].copy()  # [PB, C]
        slot = np.arange(C)[None, :]
        ids_t[slot >= n1[:, t : t + 1]] = ctx_item_bf16.shape[0] - 1
        pref[:, offs[t] : offs[t + 1], :] = ctx_item_bf16[ids_t]

    uvec = user_emb[user_ids[ob].reshape(-1)].reshape(PB, nt * D)
    pn = np.empty((PB, nt, 2, D), np.float32)
    pn[:, :, 0, :] = ctx_item[pos_ids[ob]]
    pn[:, :, 1, :] = ctx_item[neg_ids[ob]]
    # negated: the kernel adds it as an ACT bias to the exp-sum
    padc = (n1 - np.asarray(cap, np.float32)[None, :]).astype(np.float32)

    return {
        "pref": np.ascontiguousarray(pref.reshape(PB, ctot * D)),
        "uvec": np.ascontiguousarray(uvec.astype(np.float32)),
        "pnvec": np.ascontiguousarray(pn.reshape(PB, nt * 2 * D)),
        "padc": padc,
    }


def plan_order(n_prefs_core, cap):
    """Assign the core's bc elements to (partition, tile) slots so each
    element lands in a tile with cap >= n+1. Returns order [bc] or None."""
    bc = n_prefs_core.shape[0]
    nt = bc // PB
    idx = np.argsort(n_prefs_core, kind="stable")  # ascending n
    order = np.empty(bc, dtype=np.int64)
    tile_order = np.argsort(np.asarray(cap), kind="stable")
    ok = True
    pos = 0
    for t in tile_order:
        members = idx[pos : pos + PB]
        if (n_prefs_core[members] + 1 > cap[t]).any():
            ok = False
        order[t::nt] = members
        pos += PB
    if not ok:
        return None
    return order


def default_caps(nt):
    # quantiles of Uniform{1..49} n_prefs + slack, rounded up to x4 for
    # clean reduction trees; DESCENDING so the big tiles start first and
    # the small ones fill the pipeline tail
    qs = []
    for i in range(nt):
        c = int(np.ceil(2 + 48.0 * (i + 1) / nt)) + 2
        c = min(P, ((c + 3) // 4) * 4)
        qs.append(c)
    return tuple(qs)


def kernel(user_emb, item_emb, user_ids, pos_ids, neg_ids, pref_ids, n_prefs,
           _trace=False):
    user_emb = np.ascontiguousarray(np.asarray(user_emb, np.float32))
    item_emb = np.asarray(item_emb, np.float32)
    ctx_item = np.concatenate([item_emb, np.zeros((1, D), np.float32)], axis=0)
    import ml_dtypes

    ctx_item_bf16 = ctx_item.astype(ml_dtypes.bfloat16)

    user_ids = np.asarray(user_ids)
    pos_ids = np.asarray(pos_ids)
    neg_ids = np.asarray(neg_ids)
    pref_ids = np.asarray(pref_ids)
    n_prefs = np.asarray(n_prefs, np.float32)

    nt = BC // PB
    cap = default_caps(nt)

    orders = []
    feasible = True
    for c in range(N_CORES):
        sl = slice(c * BC, (c + 1) * BC)
        o = plan_order(n_prefs[sl], cap)
        if o is None:
            feasible = False
            break
        orders.append(o)
    if not feasible:
        cap = (P,) * nt
        orders = [plan_order(n_prefs[c * BC : (c + 1) * BC], cap) for c in range(N_CORES)]

    nc = _get_bass(BC, cap)

    in_maps = []
    for c in range(N_CORES):
        sl = slice(c * BC, (c + 1) * BC)
        in_maps.append(
            prep_core(
                user_emb,
                ctx_item_bf16,
                ctx_item,
                user_ids[sl],
                pos_ids[sl],
                neg_ids[sl],
                pref_ids[sl],
                n_prefs[sl],
                cap,
                orders[c],
            )
        )

    res = run_bass_kernel_spmd(
        nc, in_maps, core_ids=list(range(N_CORES)), trace=_trace
    )

    out = np.empty((2, B), dtype=np.float32)
    for c in range(N_CORES):
        r = np.asarray(res.results[c]["out"])  # [PB, 2*nt]
        r = r.reshape(PB, 2, nt)  # [p, s, t]
        flat = r.transpose(1, 0, 2).reshape(2, BC)  # [(s), p*nt+t]
        out[:, c * BC : (c + 1) * BC][:, orders[c]] = flat
    if _trace:
        return out, res
    return out



# revision 5
# speedup vs baseline: 2.4881x; 2.4881x over previous
"""AttCML distributed Bass kernel for 8 TRN2 NeuronCores — TensorEngine version.

Sharding: data-parallel over batch (16384 samples assigned freely to cores).

Both attention contractions run on the (otherwise idle) PE array instead of
DVE, which was the baseline bottleneck at ~88% busy:

  - samples are packed into "quads": 128 partition rows = s samples x c pref
    slots, with geometry pools c in {16, 32, 64} chosen per sample so that
    n+1 <= c.  A "group" is the set of quads whose target columns fill one
    128-column PSUM bank (Q = 128 / (2s) quads).
  - stage A (scores):  per quad  w[(k,j), (k',t)] = prefT^T @ tgt   where
    prefT [d=128, 128 slots] is the fp8 stationary (FWL) and the 2s target
    columns stream.  All Q quads of a group land side by side in one PSUM
    bank -> w-group [128, 128].
  - softmax: DVE adds a per-geometry block mask (-30 off-block), ACT exps
    to fp8 (off-block underflows to exact 0, so cross-sample terms vanish).
  - denominator: one matmul with an all-ones [128,128] stationary gives
    S broadcast across all partitions; a second 1-partition matmul
    accumulates the host-computed pad-count correction.  DVE reciprocal.
  - stage C (pooling): per quad  r^T[d, (k,t)] = prefQ^T @ e  with the
    fp8 slot-major pref as stationary and the masked e columns streaming.
  - distances: r^T * (1/S) + diff0^T (host-precomputed u - tgt), squared on
    ACT, then summed over d (partitions) by a ones-column matmul; the
    [1, 512] distance rows are copied out per 4-group superblock.

Pad slots are zero rows: they add exp(0)=1 to the raw denominator (fixed by
the padc matmul) and 0 to the pooled vector — exact reference semantics.
"""

import numpy as np
from contextlib import ExitStack

try:
    import concourse  # noqa: F401
except ImportError:  # pragma: no cover
    import sys

    for _p in ("/opt/trn_rl_repo", "/root/.axon_site/_ro/trn_rl_repo"):
        if _p not in sys.path:
            sys.path.insert(0, _p)

import ml_dtypes
import concourse.bacc as bacc
import concourse.bass as bass
import concourse.tile as tile
from concourse import mybir
from concourse.bass_utils import run_bass_kernel_spmd

F32 = mybir.dt.float32
BF16 = mybir.dt.bfloat16
FP8 = mybir.dt.float8e3  # e3m4
ALU = mybir.AluOpType
ACTF = mybir.ActivationFunctionType

FP8NP = ml_dtypes.float8_e3m4
BF16NP = ml_dtypes.bfloat16

D = 128
P = 50
N_CORES = 8
B = 16384

# geometry pools: (c slots/sample, s samples/quad, Q quads/group, NG groups/core)
GEOS = ((16, 8, 8, 10), (32, 4, 16, 11), (64, 2, 32, 13))
NG_TOT = sum(g[3] for g in GEOS)  # 34 groups/core
NQ_TOT = sum(g[2] * g[3] for g in GEOS)  # 672 quads/core
NCOL = NG_TOT * 128  # 4352 target cols/core
SLOT_COLS = NQ_TOT * 128  # 86016 slots/core
SB = 4  # groups per distance superblock
MASKVAL = -30.0

# flat group table: (geo_idx, c, s, Q, quad_base)
GROUPS = []
_qb = 0
for _gi, (_c, _s, _Q, _NG) in enumerate(GEOS):
    for _ in range(_NG):
        GROUPS.append((_gi, _c, _s, _Q, _qb))
        _qb += _Q


def build_bass():
    nc = bacc.Bacc(
        "TRN2",
        target_bir_lowering=False,
        debug=False,
        enable_asserts=False,
        num_devices=N_CORES,
    )

    prefT_in = nc.declare_dram_parameter("prefT", [128, SLOT_COLS], FP8, isOutput=False)
    prefQ_in = nc.declare_dram_parameter("prefQ", [128, SLOT_COLS], FP8, isOutput=False)
    tgt_in = nc.declare_dram_parameter("tgt", [128, NCOL], FP8, isOutput=False)
    d0_in = nc.declare_dram_parameter("d0", [128, NCOL], BF16, isOutput=False)
    padc_in = nc.declare_dram_parameter("padc", [1, NCOL], BF16, isOutput=False)
    maskb_in = nc.declare_dram_parameter(
        "maskb", [128, 128 * len(GEOS)], BF16, isOutput=False
    )
    ones8_in = nc.declare_dram_parameter("ones8", [128, 128], FP8, isOutput=False)
    onesr_in = nc.declare_dram_parameter("onesr", [1, 128], BF16, isOutput=False)
    onesc_in = nc.declare_dram_parameter("onesc", [128, 1], BF16, isOutput=False)
    out_d = nc.declare_dram_parameter("out", [1, NCOL], F32, isOutput=True)

    with tile.TileContext(nc) as tc, ExitStack() as ctx:
        ctx.enter_context(
            nc.allow_low_precision(reason="fp8/bf16 pipeline validated vs reference")
        )
        consts = ctx.enter_context(tc.tile_pool(name="consts", bufs=1))
        pT_pool = ctx.enter_context(tc.tile_pool(name="pT", bufs=4))
        pQ_pool = ctx.enter_context(tc.tile_pool(name="pQ", bufs=4))
        tg_pool = ctx.enter_context(tc.tile_pool(name="tg", bufs=4))
        d0_pool = ctx.enter_context(tc.tile_pool(name="d0", bufs=4))
        sm_pool = ctx.enter_context(tc.tile_pool(name="sm", bufs=2))
        q2_pool = ctx.enter_context(tc.tile_pool(name="q2", bufs=2))
        w_ps = ctx.enter_context(
            tc.tile_pool(name="wps", bufs=2, space=bass.MemorySpace.PSUM)
        )
        s_ps = ctx.enter_context(
            tc.tile_pool(name="sps", bufs=2, space=bass.MemorySpace.PSUM)
        )
        r_ps = ctx.enter_context(
            tc.tile_pool(name="rps", bufs=2, space=bass.MemorySpace.PSUM)
        )
        o_ps = ctx.enter_context(
            tc.tile_pool(name="ops", bufs=2, space=bass.MemorySpace.PSUM)
        )

        maskb = consts.tile([128, 128 * len(GEOS)], BF16)
        nc.sync.dma_start(maskb[:], maskb_in[:])
        ones8 = consts.tile([128, 128], FP8)
        nc.sync.dma_start(ones8[:], ones8_in[:])
        onesr = consts.tile([1, 128], BF16)
        nc.sync.dma_start(onesr[:], onesr_in[:])
        onesc = consts.tile([128, 1], BF16)
        nc.sync.dma_start(onesc[:], onesc_in[:])
        padcr = consts.tile([1, NCOL], BF16)
        nc.sync.dma_start(padcr[:], padc_in[:])
        out_s = consts.tile([1, NCOL], F32)

        dmas = [None] * NG_TOT

        def prefetch(g):
            gi, c, s, Q, qb = GROUPS[g]
            pT = pT_pool.tile([128, Q * 128], FP8, tag="pT")
            nc.sync.dma_start(pT[:], prefT_in[:, qb * 128 : (qb + Q) * 128])
            pQ = pQ_pool.tile([128, Q * 128], FP8, tag="pQ")
            nc.sync.dma_start(pQ[:], prefQ_in[:, qb * 128 : (qb + Q) * 128])
            tg = tg_pool.tile([128, 128], FP8, tag="tg")
            nc.sync.dma_start(tg[:], tgt_in[:, g * 128 : (g + 1) * 128])
            d0 = d0_pool.tile([128, 128], BF16, tag="d0")
            nc.sync.dma_start(d0[:], d0_in[:, g * 128 : (g + 1) * 128])
            dmas[g] = (pT, pQ, tg, d0)

        def stage_a(g):
            gi, c, s, Q, qb = GROUPS[g]
            pT, pQ, tg, d0 = dmas[g]
            m = 2 * s
            wps = w_ps.tile([128, 512], F32, tag="w")
            for q in range(Q):
                nc.tensor.matmul(
                    wps[:, q * m : (q + 1) * m],
                    pT[:, q * 128 : (q + 1) * 128],
                    tg[:, q * m : (q + 1) * m],
                )
            return wps

        # superblock state for the distance reduction
        q2sb = {"tile": None, "g0": 0}

        def finish(g, wps):
            gi, c, s, Q, qb = GROUPS[g]
            pT, pQ, tg, d0 = dmas[g]
            m = 2 * s

            wm = sm_pool.tile([128, 128], BF16, tag="wm")
            nc.vector.tensor_tensor(
                out=wm[:],
                in0=wps[:, :128],
                in1=maskb[:, gi * 128 : (gi + 1) * 128],
                op=ALU.add,
            )
            ee = sm_pool.tile([128, 128], FP8, tag="ee")
            nc.scalar.activation(ee[:], wm[:], ACTF.Exp)

            sps = s_ps.tile([128, 512], F32, tag="s")
            nc.tensor.matmul(sps[:, :128], ones8[:], ee[:], start=True, stop=False)
            nc.tensor.matmul(
                sps[:, :128],
                onesr[:],
                padcr[:, g * 128 : (g + 1) * 128],
                start=False,
                stop=True,
                skip_group_check=True,
            )
            rs = sm_pool.tile([128, 128], BF16, tag="rs")
            nc.vector.reciprocal(rs[:], sps[:, :128])

            rps = r_ps.tile([128, 512], F32, tag="r")
            for q in range(Q):
                nc.tensor.matmul(
                    rps[:, q * m : (q + 1) * m],
                    pQ[:, q * 128 : (q + 1) * 128],
                    ee[:, q * m : (q + 1) * m],
                )

            rm = sm_pool.tile([128, 128], BF16, tag="rm")
            nc.vector.tensor_tensor(out=rm[:], in0=rps[:, :128], in1=rs[:], op=ALU.mult)
            qv = sm_pool.tile([128, 128], BF16, tag="qv")
            nc.vector.tensor_tensor(out=qv[:], in0=rm[:], in1=d0[:], op=ALU.add)

            if g % SB == 0:
                q2sb["tile"] = q2_pool.tile(
                    [128, SB * 128], BF16, tag="q2", name="q2t"
                )
                q2sb["g0"] = g
            k = g - q2sb["g0"]
            nc.scalar.activation(
                q2sb["tile"][:, k * 128 : (k + 1) * 128], qv[:], ACTF.Square
            )
            if g == NG_TOT - 1 or g % SB == SB - 1:
                width = (k + 1) * 128
                ops = o_ps.tile([1, 512], F32, tag="o")
                nc.tensor.matmul(ops[:, :width], onesc[:], q2sb["tile"][:, :width])
                nc.scalar.copy(
                    out_s[:, q2sb["g0"] * 128 : q2sb["g0"] * 128 + width],
                    ops[:, :width],
                )

        # software pipeline: PE runs A(g+1) while softmax(g) is on DVE/ACT
        for g in range(min(3, NG_TOT)):
            prefetch(g)
        wcur = stage_a(0)
        for g in range(NG_TOT):
            if g + 3 < NG_TOT:
                prefetch(g + 3)
            wnext = stage_a(g + 1) if g + 1 < NG_TOT else None
            finish(g, wcur)
            wcur = wnext

        nc.sync.dma_start(out_d[:], out_s[:])

    nc.compile()
    return nc


_CACHE = {}


def _get_bass():
    if "nc" not in _CACHE:
        _CACHE["nc"] = build_bass()
    return _CACHE["nc"]


def _build_masks():
    maskb = np.zeros((128, 128 * len(GEOS)), np.float32)
    for gi, (c, s, Q, NG) in enumerate(GEOS):
        m = 2 * s
        rows = np.arange(128) // c  # sample block of each slot row
        cols = np.arange(128)
        colblk = (cols % m) // 2  # sample block of each target col
        blk = rows[:, None] != colblk[None, :]
        maskb[:, gi * 128 : (gi + 1) * 128] = np.where(blk, MASKVAL, 0.0)
    return maskb.astype(BF16NP)


def prep_core(ctx32, ctx8, user_emb, user_ids, pos_ids, neg_ids, pref_ids, v, samples):
    """Build one core's input map.

    samples: dict geo_idx -> int64 array of global sample indices (len <= NG*64)
    Returns (in_map, colsamp, colt) for output unscrambling.
    """
    ZERO = ctx8.shape[0] - 1

    sid = np.full((NQ_TOT, 128), ZERO, np.int64)
    colsamp = np.full(NCOL, -1, np.int64)
    colt = np.zeros(NCOL, np.int64)
    tid = np.full(NCOL, ZERO, np.int64)
    uid = np.zeros(NCOL, np.int64)
    valid = np.zeros(NCOL, bool)
    padc = np.zeros(NCOL, np.float32)

    g0 = 0
    qb0 = 0
    for gi, (c, s, Q, NG) in enumerate(GEOS):
        samp = samples[gi]
        n = samp.shape[0]
        assert n <= NG * 64
        r = np.arange(n)
        g_rel = r // 64
        q = (r % 64) // s
        k = (r % 64) % s
        quad = qb0 + g_rel * Q + q
        rowbase = k * c

        # pref slot ids [n, c], padded with ZERO where j >= v
        cp = min(c, P)
        ids = np.full((n, c), ZERO, np.int64)
        ids[:, :cp] = pref_ids[samp, :cp]
        jj = np.arange(c)[None, :]
        ids[jj >= v[samp][:, None]] = ZERO
        sid[quad[:, None], rowbase[:, None] + jj] = ids

        colbase = (g0 + g_rel) * 128 + q * (2 * s) + 2 * k
        for t, t_ids in ((0, pos_ids), (1, neg_ids)):
            cc = colbase + t
            colsamp[cc] = samp
            colt[cc] = t
            tid[cc] = t_ids[samp]
            uid[cc] = user_ids[samp]
            valid[cc] = True
            padc[cc] = -(c - v[samp])

        g0 += NG
        qb0 += NG * Q

    # pref tensors: gather once, emit both layouts
    g8 = ctx8[sid]  # [NQ_TOT, 128, 128] fp8
    prefQ = np.ascontiguousarray(g8.transpose(1, 0, 2)).reshape(128, SLOT_COLS)
    prefT = np.ascontiguousarray(g8.transpose(2, 0, 1)).reshape(128, SLOT_COLS)

    tgt = np.ascontiguousarray(ctx8[tid].T)  # [128, NCOL] fp8
    d0f = user_emb[uid] - ctx32[tid]  # [NCOL, 128] f32
    d0f[~valid] = 0.0
    d0T = np.ascontiguousarray(d0f.T).astype(BF16NP)

    in_map = {
        "prefT": prefT,
        "prefQ": prefQ,
        "tgt": tgt,
        "d0": d0T,
        "padc": padc.astype(BF16NP).reshape(1, NCOL),
        "maskb": _MASKB,
        "ones8": np.ones((128, 128), FP8NP),
        "onesr": np.ones((1, 128), BF16NP),
        "onesc": np.ones((128, 1), BF16NP),
    }
    return in_map, colsamp, colt, valid


_MASKB = _build_masks()


def kernel(user_emb, item_emb, user_ids, pos_ids, neg_ids, pref_ids, n_prefs,
           _trace=False):
    user_emb = np.ascontiguousarray(np.asarray(user_emb, np.float32))
    item_emb = np.asarray(item_emb, np.float32)
    user_ids = np.asarray(user_ids).astype(np.int64)
    pos_ids = np.asarray(pos_ids).astype(np.int64)
    neg_ids = np.asarray(neg_ids).astype(np.int64)
    pref_ids = np.asarray(pref_ids).astype(np.int64)
    n_prefs = np.asarray(n_prefs, np.float32)

    ctx32 = np.concatenate([item_emb, np.zeros((1, D), np.float32)], axis=0)
    ctx8 = ctx32.astype(FP8NP)

    v = n_prefs.astype(np.int64) + 1  # valid slot counts

    # pool assignment with spill-up (smallest feasible c first)
    nb = user_ids.shape[0]
    rem = np.arange(nb)
    pool_of = {}
    for gi, (c, s, Q, NG) in enumerate(GEOS):
        cap = NG * 64 * N_CORES
        elig = rem[v[rem] <= c]
        chosen = elig[:cap]
        pool_of[gi] = chosen
        rem = np.setdiff1d(rem, chosen, assume_unique=True)
    assert rem.size == 0, "geometry pool capacity overflow"

    nc = _get_bass()

    in_maps = []
    unscr = []
    for core in range(N_CORES):
        samples = {
            gi: np.array_split(pool_of[gi], N_CORES)[core] for gi in range(len(GEOS))
        }
        im, colsamp, colt, valid = prep_core(
            ctx32, ctx8, user_emb, user_ids, pos_ids, neg_ids, pref_ids, v, samples
        )
        in_maps.append(im)
        unscr.append((colsamp, colt, valid))

    res = run_bass_kernel_spmd(
        nc, in_maps, core_ids=list(range(N_CORES)), trace=_trace
    )

    out = np.empty((2, nb), dtype=np.float32)
    for core in range(N_CORES):
        r = np.asarray(res.results[core]["out"]).reshape(NCOL)
        colsamp, colt, valid = unscr[core]
        out[colt[valid], colsamp[valid]] = r[valid]
    if _trace:
        return out, res
    return out


# revision 6
# speedup vs baseline: 2.8074x; 1.1284x over previous
"""AttCML distributed Bass kernel for 8 TRN2 NeuronCores — TensorEngine version.

Sharding: data-parallel over batch (16384 samples assigned freely to cores).

Both attention contractions run on the (otherwise idle) PE array instead of
DVE, which was the baseline bottleneck at ~88% busy:

  - samples are packed into "quads": 128 partition rows = s samples x c pref
    slots, geometry pools c in {16, 32, 42, 64} chosen so n+1 <= c.
  - stage A (scores):  per quad  w[(k,j), (k',t)] = prefT^T @ tgt   with the
    fp8 d-major pref as FWL stationary and 2s target columns streaming; Q
    quads fill 128 PSUM columns per group, 4 groups share one bank
    ("superblock" [128, 512]) so softmax runs as single wide DVE/ACT ops.
  - softmax: DVE adds a per-group block-mask bias (-30 off-block), ACT exps
    to fp8 (off-block underflows to exact 0, killing cross-sample terms).
  - denominator: a matmul with an all-ones [128,128] stationary gives S
    broadcast across partitions; a 1-partition matmul accumulates the
    host-side pad-count correction; DVE reciprocal.
  - stage C (pooling): per quad  r^T[d, (k,t)] = prefQ^T @ e  with the fp8
    slot-major pref as FWL stationary and masked e columns streaming.
  - distances: r^T * (1/S)  (DVE) + diff0^T (GpSimd; host-precomputed
    u - tgt), squared on ACT, summed over d by a ones-column matmul,
    [1, 512] rows copied out per superblock.

Pad slots are zero rows: they add exp(0)=1 to the raw denominator (fixed by
the padc matmul) and 0 to the pooled vector — exact reference semantics.
"""

import numpy as np
from contextlib import ExitStack

try:
    import concourse  # noqa: F401
except ImportError:  # pragma: no cover
    import sys

    for _p in ("/opt/trn_rl_repo", "/root/.axon_site/_ro/trn_rl_repo"):
        if _p not in sys.path:
            sys.path.insert(0, _p)

import ml_dtypes
import concourse.bacc as bacc
import concourse.bass as bass
import concourse.tile as tile
from concourse import mybir
from concourse.bass_utils import run_bass_kernel_spmd

F32 = mybir.dt.float32
BF16 = mybir.dt.bfloat16
FP8 = mybir.dt.float8e3  # e3m4
ALU = mybir.AluOpType
ACTF = mybir.ActivationFunctionType

FP8NP = ml_dtypes.float8_e3m4
BF16NP = ml_dtypes.bfloat16

D = 128
P = 50
N_CORES = 8
B = 16384

# geometry pools: (c slots/sample, s samples/quad, Q quads/group, NG groups/core)
GEOS = ((16, 8, 8, 10), (32, 4, 16, 11), (42, 3, 21, 7), (64, 2, 32, 6))
NG_TOT = sum(g[3] for g in GEOS)  # 34 groups/core
SB = 4  # groups per superblock (one 512-col PSUM bank)
MASKVAL = -30.0

# interleave pools so DMA-heavy c=64 groups spread across the kernel
_items = []
for _gi, (_c, _s, _Q, _NG) in enumerate(GEOS):
    for _k in range(_NG):
        _items.append(((_k + 0.5) / _NG, _gi))
_items.sort()
# GROUPS[g] = (geo_idx, c, s, Q, quad_base)
GROUPS = []
POOL_GROUPS = {gi: [] for gi in range(len(GEOS))}
_qb = 0
for _, _gi in _items:
    _c, _s, _Q, _NG = GEOS[_gi]
    POOL_GROUPS[_gi].append(len(GROUPS))
    GROUPS.append((_gi, _c, _s, _Q, _qb))
    _qb += _Q
NQ_TOT = _qb  # 595 quads/core
NCOL = NG_TOT * 128  # 4352
SLOT_COLS = NQ_TOT * 128  # 76160

# superblocks: (first group, number of groups)
SBS = [(g0, min(SB, NG_TOT - g0)) for g0 in range(0, NG_TOT, SB)]


def build_bass():
    nc = bacc.Bacc(
        "TRN2",
        target_bir_lowering=False,
        debug=False,
        enable_asserts=False,
        num_devices=N_CORES,
    )

    prefT_in = nc.declare_dram_parameter("prefT", [128, SLOT_COLS], FP8, isOutput=False)
    prefQ_in = nc.declare_dram_parameter("prefQ", [128, SLOT_COLS], FP8, isOutput=False)
    tgt_in = nc.declare_dram_parameter("tgt", [128, NCOL], FP8, isOutput=False)
    d0_in = nc.declare_dram_parameter("d0", [128, NCOL], BF16, isOutput=False)
    padc_in = nc.declare_dram_parameter("padc", [1, NCOL], BF16, isOutput=False)
    maskb_in = nc.declare_dram_parameter("maskb", [128, NCOL], BF16, isOutput=False)
    ones8_in = nc.declare_dram_parameter("ones8", [128, 128], FP8, isOutput=False)
    onesr_in = nc.declare_dram_parameter("onesr", [1, 128], BF16, isOutput=False)
    onesc_in = nc.declare_dram_parameter("onesc", [128, 1], BF16, isOutput=False)
    out_d = nc.declare_dram_parameter("out", [1, NCOL], F32, isOutput=True)

    with tile.TileContext(nc) as tc, ExitStack() as ctx:
        ctx.enter_context(
            nc.allow_low_precision(reason="fp8/bf16 pipeline validated vs reference")
        )
        consts = ctx.enter_context(tc.tile_pool(name="consts", bufs=1))
        pT_pool = ctx.enter_context(tc.tile_pool(name="pT", bufs=3))
        pQ_pool = ctx.enter_context(tc.tile_pool(name="pQ", bufs=3))
        tg_pool = ctx.enter_context(tc.tile_pool(name="tg", bufs=3))
        d0_pool = ctx.enter_context(tc.tile_pool(name="d0", bufs=3))
        sm_pool = ctx.enter_context(tc.tile_pool(name="sm", bufs=2))
        q2_pool = ctx.enter_context(tc.tile_pool(name="q2", bufs=2))
        w_ps = ctx.enter_context(
            tc.tile_pool(name="wps", bufs=2, space=bass.MemorySpace.PSUM)
        )
        s_ps = ctx.enter_context(
            tc.tile_pool(name="sps", bufs=2, space=bass.MemorySpace.PSUM)
        )
        r_ps = ctx.enter_context(
            tc.tile_pool(name="rps", bufs=2, space=bass.MemorySpace.PSUM)
        )
        o_ps = ctx.enter_context(
            tc.tile_pool(name="ops", bufs=2, space=bass.MemorySpace.PSUM)
        )

        maskb = consts.tile([128, NCOL], BF16)
        nc.sync.dma_start(maskb[:], maskb_in[:])
        ones8 = consts.tile([128, 128], FP8)
        nc.sync.dma_start(ones8[:], ones8_in[:])
        onesr = consts.tile([1, 128], BF16)
        nc.sync.dma_start(onesr[:], onesr_in[:])
        onesc = consts.tile([128, 1], BF16)
        nc.sync.dma_start(onesc[:], onesc_in[:])
        padcr = consts.tile([1, NCOL], BF16)
        nc.sync.dma_start(padcr[:], padc_in[:])
        out_s = consts.tile([1, NCOL], F32)

        sb_tiles = [None] * len(SBS)

        def prefetch(sb):
            g0, ng = SBS[sb]
            wid = ng * 128
            qb0 = GROUPS[g0][4]
            qb1 = GROUPS[g0 + ng - 1][4] + GROUPS[g0 + ng - 1][3]
            pT = pT_pool.tile([128, (qb1 - qb0) * 128], FP8, tag="pT", name="pT")
            nc.sync.dma_start(pT[:], prefT_in[:, qb0 * 128 : qb1 * 128])
            pQ = pQ_pool.tile([128, (qb1 - qb0) * 128], FP8, tag="pQ", name="pQ")
            nc.sync.dma_start(pQ[:], prefQ_in[:, qb0 * 128 : qb1 * 128])
            tg = tg_pool.tile([128, wid], FP8, tag="tg", name="tg")
            nc.sync.dma_start(tg[:], tgt_in[:, g0 * 128 : g0 * 128 + wid])
            d0 = d0_pool.tile([128, wid], BF16, tag="d0", name="d0")
            nc.sync.dma_start(d0[:], d0_in[:, g0 * 128 : g0 * 128 + wid])
            sb_tiles[sb] = (pT, pQ, tg, d0, qb0, wid)

        def stage_a(sb):
            g0, ng = SBS[sb]
            pT, pQ, tg, d0, qb0, wid = sb_tiles[sb]
            wps = w_ps.tile([128, 512], F32, tag="w", name="wps")
            for g in range(g0, g0 + ng):
                gi, c, s, Q, qb = GROUPS[g]
                m = 2 * s
                co = (g - g0) * 128
                for q in range(Q):
                    nc.tensor.matmul(
                        wps[:, co + q * m : co + (q + 1) * m],
                        pT[:, (qb - qb0 + q) * 128 : (qb - qb0 + q + 1) * 128],
                        tg[:, co + q * m : co + (q + 1) * m],
                    )
            return wps

        pend_dist = [None]

        def emit_dist():
            if pend_dist[0] is None:
                return
            q2, g0, wid = pend_dist[0]
            pend_dist[0] = None
            ops = o_ps.tile([1, 512], F32, tag="o", name="ops")
            nc.tensor.matmul(ops[:, :wid], onesc[:], q2[:, :wid])
            nc.scalar.copy(out_s[:, g0 * 128 : g0 * 128 + wid], ops[:, :wid])

        def finish(sb, wps):
            g0, ng = SBS[sb]
            pT, pQ, tg, d0, qb0, wid = sb_tiles[sb]
            c0 = g0 * 128

            wm = sm_pool.tile([128, 512], BF16, tag="wm", name="wm")
            nc.vector.tensor_tensor(
                out=wm[:, :wid],
                in0=wps[:, :wid],
                in1=maskb[:, c0 : c0 + wid],
                op=ALU.add,
            )
            ee = sm_pool.tile([128, 512], FP8, tag="ee", name="ee")
            nc.scalar.activation(ee[:, :wid], wm[:, :wid], ACTF.Exp)

            sps = s_ps.tile([128, 512], F32, tag="s", name="sps")
            nc.tensor.matmul(
                sps[:, :wid], ones8[:], ee[:, :wid], start=True, stop=False
            )
            nc.tensor.matmul(
                sps[:, :wid],
                onesr[:],
                padcr[:, c0 : c0 + wid],
                start=False,
                stop=True,
                skip_group_check=True,
            )
            rs = sm_pool.tile([128, 512], BF16, tag="rs", name="rs")
            nc.vector.reciprocal(rs[:, :wid], sps[:, :wid])

            rps = r_ps.tile([128, 512], F32, tag="r", name="rps")
            for g in range(g0, g0 + ng):
                gi, c, s, Q, qb = GROUPS[g]
                m = 2 * s
                co = (g - g0) * 128
                for q in range(Q):
                    nc.tensor.matmul(
                        rps[:, co + q * m : co + (q + 1) * m],
                        pQ[:, (qb - qb0 + q) * 128 : (qb - qb0 + q + 1) * 128],
                        ee[:, co + q * m : co + (q + 1) * m],
                    )

            rm = sm_pool.tile([128, 512], BF16, tag="rm", name="rm")
            nc.vector.tensor_tensor(
                out=rm[:, :wid], in0=rps[:, :wid], in1=rs[:, :wid], op=ALU.mult
            )
            qv = sm_pool.tile([128, 512], BF16, tag="qv", name="qv")
            nc.gpsimd.tensor_add(qv[:, :wid], rm[:, :wid], d0[:, :wid])
            q2 = q2_pool.tile([128, 512], BF16, tag="q2", name="q2")
            nc.scalar.activation(q2[:, :wid], qv[:, :wid], ACTF.Square)
            pend_dist[0] = (q2, g0, wid)

        # software pipeline at superblock granularity
        NSB = len(SBS)
        for sb in range(min(2, NSB)):
            prefetch(sb)
        wcur = stage_a(0)
        for sb in range(NSB):
            if sb + 2 < NSB:
                prefetch(sb + 2)
            wnext = stage_a(sb + 1) if sb + 1 < NSB else None
            emit_dist()  # previous superblock's distance reduction
            finish(sb, wcur)
            wcur = wnext
        emit_dist()

        nc.sync.dma_start(out_d[:], out_s[:])

    nc.compile()
    return nc


_CACHE = {}


def _get_bass():
    if "nc" not in _CACHE:
        _CACHE["nc"] = build_bass()
    return _CACHE["nc"]


def _build_masks():
    """Per-group block-mask bias [128, NCOL]; dead cols fully masked."""
    mb = np.full((128, NCOL), MASKVAL, np.float32)
    rows = np.arange(128)
    for g, (gi, c, s, Q, qb) in enumerate(GROUPS):
        m = 2 * s
        rblk = rows // c  # sample block of each slot row (may exceed s-1)
        cols = np.arange(Q * m)
        cblk = (cols % m) // 2
        blk = rblk[:, None] == cblk[None, :]
        mb[:, g * 128 : g * 128 + Q * m] = np.where(blk, 0.0, MASKVAL)
    return np.ascontiguousarray(mb.astype(BF16NP))


_MASKB = _build_masks()


def prep_core(ctx32, ctx8, user_emb, user_ids, pos_ids, neg_ids, pref_ids, v, samples):
    """Build one core's input map.

    samples: dict geo_idx -> int64 array of global sample indices
    Returns (in_map, colsamp, colt, valid) for output unscrambling.
    """
    ZERO = ctx8.shape[0] - 1

    sid = np.full((NQ_TOT, 128), ZERO, np.int64)
    colsamp = np.full(NCOL, -1, np.int64)
    colt = np.zeros(NCOL, np.int64)
    tid = np.full(NCOL, ZERO, np.int64)
    uid = np.zeros(NCOL, np.int64)
    valid = np.zeros(NCOL, bool)
    padc = np.zeros(NCOL, np.float32)

    # dead cols (beyond Q*m within each group's 128): S_raw = 0 -> force S = 1
    for g, (gi, c, s, Q, qb) in enumerate(GROUPS):
        if Q * 2 * s < 128:
            padc[g * 128 + Q * 2 * s : (g + 1) * 128] = 1.0

    for gi, (c, s, Q, NG) in enumerate(GEOS):
        samp = samples[gi]
        n = samp.shape[0]
        spg = Q * s
        assert n <= NG * spg
        r = np.arange(n)
        g_abs = np.asarray(POOL_GROUPS[gi])[r // spg]
        q = (r % spg) // s
        k = (r % spg) % s
        quad = np.array([GROUPS[g][4] for g in g_abs]) + q
        rowbase = k * c

        cp = min(c, P)
        ids = np.full((n, c), ZERO, np.int64)
        ids[:, :cp] = pref_ids[samp, :cp]
        jj = np.arange(c)[None, :]
        ids[jj >= v[samp][:, None]] = ZERO
        sid[quad[:, None], rowbase[:, None] + jj] = ids

        colbase = g_abs * 128 + q * (2 * s) + 2 * k
        for t, t_ids in ((0, pos_ids), (1, neg_ids)):
            cc = colbase + t
            colsamp[cc] = samp
            colt[cc] = t
            tid[cc] = t_ids[samp]
            uid[cc] = user_ids[samp]
            valid[cc] = True
            padc[cc] = -(c - v[samp])

    # pref tensors: gather once, emit both layouts
    g8 = ctx8[sid]  # [NQ_TOT, 128, 128] fp8
    prefQ = np.ascontiguousarray(g8.transpose(1, 0, 2)).reshape(128, SLOT_COLS)
    prefT = np.ascontiguousarray(g8.transpose(2, 0, 1)).reshape(128, SLOT_COLS)

    tgt = np.ascontiguousarray(ctx8[tid].T)  # [128, NCOL] fp8
    d0f = user_emb[uid] - ctx32[tid]  # [NCOL, 128] f32
    d0f[~valid] = 0.0
    d0T = np.ascontiguousarray(d0f.T).astype(BF16NP)

    in_map = {
        "prefT": prefT,
        "prefQ": prefQ,
        "tgt": tgt,
        "d0": d0T,
        "padc": padc.astype(BF16NP).reshape(1, NCOL),
        "maskb": _MASKB,
        "ones8": np.ones((128, 128), FP8NP),
        "onesr": np.ones((1, 128), BF16NP),
        "onesc": np.ones((128, 1), BF16NP),
    }
    return in_map, colsamp, colt, valid


def kernel(user_emb, item_emb, user_ids, pos_ids, neg_ids, pref_ids, n_prefs,
           _trace=False):
    user_emb = np.ascontiguousarray(np.asarray(user_emb, np.float32))
    item_emb = np.asarray(item_emb, np.float32)
    user_ids = np.asarray(user_ids).astype(np.int64)
    pos_ids = np.asarray(pos_ids).astype(np.int64)
    neg_ids = np.asarray(neg_ids).astype(np.int64)
    pref_ids = np.asarray(pref_ids).astype(np.int64)
    n_prefs = np.asarray(n_prefs, np.float32)

    ctx32 = np.concatenate([item_emb, np.zeros((1, D), np.float32)], axis=0)
    ctx8 = ctx32.astype(FP8NP)

    v = n_prefs.astype(np.int64) + 1  # valid slot counts

    # pool assignment with spill-up (smallest feasible c first)
    nb = user_ids.shape[0]
    rem = np.arange(nb)
    pool_of = {}
    for gi, (c, s, Q, NG) in enumerate(GEOS):
        cap = NG * Q * s * N_CORES
        elig = rem[v[rem] <= c]
        chosen = elig[:cap]
        pool_of[gi] = chosen
        rem = np.setdiff1d(rem, chosen, assume_unique=True)
    assert rem.size == 0, "geometry pool capacity overflow"

    nc = _get_bass()

    in_maps = []
    unscr = []
    for core in range(N_CORES):
        samples = {
            gi: np.array_split(pool_of[gi], N_CORES)[core] for gi in range(len(GEOS))
        }
        im, colsamp, colt, valid = prep_core(
            ctx32, ctx8, user_emb, user_ids, pos_ids, neg_ids, pref_ids, v, samples
        )
        in_maps.append(im)
        unscr.append((colsamp, colt, valid))

    res = run_bass_kernel_spmd(
        nc, in_maps, core_ids=list(range(N_CORES)), trace=_trace
    )

    out = np.empty((2, nb), dtype=np.float32)
    for core in range(N_CORES):
        r = np.asarray(res.results[core]["out"]).reshape(NCOL)
        colsamp, colt, valid = unscr[core]
        out[colt[valid], colsamp[valid]] = r[valid]
    if _trace:
        return out, res
    return out
